# revision 22
# baseline (speedup 1.0000x reference)
"""Trainium2 Bass kernel for nn_FIS_ImportanceAssessment.

Reference computation, per pixel (B=16, C=256, H=W=64):
    sumsq = sum_c f^2 ; sum = sum_c f
    mag   = clip(sqrt(sumsq/C), 0, 1)
    var   = clip((sumsq - sum^2/C)/(C-1), 0, 1)
    grad  = sqrt(var_clipped)               (== clip(sqrt(var), 0, 1))
    out   = sigmoid(relu([mag,var,grad] @ W1 + b1) @ W2 + b2)

Sharding: data-parallel over batch, 2 batches per core across 8 cores.

Per-core layout trick: the C-axis reduction is done on the PE with a
"block one-hot" stationary operand (float32r -> single-pass matmuls;
plain fp32 matmuls cost 4 cycles/row).  The core's 8192 pixels are
split into 16 chunks of 512; chunk g's column sums land on PSUM
partitions [8g, 8g+8), i.e. the stats arrive replicated 8x in an
(group g in 0..15, replica oh in 0..7) partition layout.  The replicas
let the whole 3->16->1 MLP run as per-partition tensor_scalar /
scalar_tensor_tensor ops (weight scalars vary only across partitions),
done twice for the two halves of the 16 hidden channels, followed by a
block-diagonal-W2 matmul that contracts the hidden channels.
"""

from contextlib import ExitStack

import numpy as np

import concourse.bacc as bacc
import concourse.bass as bass
import concourse.tile as tile
from concourse import mybir

F32 = mybir.dt.float32
F32R = mybir.dt.float32r  # TF32-style single-pass PE dtype (fp32 is 4 cyc/row)
BF16 = mybir.dt.bfloat16
AF = mybir.ActivationFunctionType
OP = mybir.AluOpType

# -------- problem geometry (hardcoded per contract) --------
B, C, H, W = 16, 256, 64, 64
NCORES = 8
B_PER_CORE = B // NCORES          # 2
PIX = B_PER_CORE * H * W          # 8192 pixels per core
NG = 16                           # pixel chunks ("groups") per core
NREP = 8                          # o-replication factor (128 / NG)
CHUNK = PIX // NG                 # 512 pixels per chunk (= 1 PSUM bank)
NHID = 16                         # MLP hidden width
NPASS = NHID // NREP              # 2 MLP passes over hidden halves

# consts_r (float32r) column layout: matmul stationary operands
ZCOL = 0          # [0:256)   block-one-hot window source (cols 128..135 = 1)
BDCOL = 256       # [256:288) block-diag W2: 256 + k*16 + g
NCONST_R = 288
# consts_f (float32) column layout: per-partition scalar operands
#   [0:6) W1 scalars 3*k+i ; [6:8) b1 ; [8] b2
NCONST_F = 16


def build_nc() -> bass.Bass:
    # Bacc (not raw Bass): its finalize() runs generate_event_semaphores,
    # which splits multi-sem waits to satisfy the 1-wait-per-instruction
    # hardware constraint that walrus codegen enforces.
    nc = bacc.Bacc()
    # float32r end-to-end for everything the PE consumes: the BIR verifier
    # requires fp32r-matmul inputs to be *produced* as float32r.
    feat = nc.dram_tensor(
        "features", [B_PER_CORE, C, H * W], F32R, kind="ExternalInput"
    )
    cst_r = nc.dram_tensor("consts_r", [128, NCONST_R], F32R, kind="ExternalInput")
    cst_h = nc.dram_tensor("consts_h", [128, 256], BF16, kind="ExternalInput")
    cst_f = nc.dram_tensor("consts_f", [128, NCONST_F], F32, kind="ExternalInput")
    out_d = nc.dram_tensor("out", [NG, CHUNK], F32, kind="ExternalOutput")

    with tile.TileContext(nc) as tc, ExitStack() as ctx:
        singles = ctx.enter_context(tc.tile_pool(name="singles", bufs=1))
        # bufs=2: both streaming rounds get fresh slots, so no x/sq DMA
        # ever carries a buffer-reuse (WAR) wait on top of its RAW wait.
        xpool = ctx.enter_context(tc.tile_pool(name="xpool", bufs=2))
        sqpool = ctx.enter_context(tc.tile_pool(name="sqpool", bufs=2))
        tailp = ctx.enter_context(tc.tile_pool(name="tailp", bufs=1))
        psump = ctx.enter_context(tc.tile_pool(name="psump", bufs=1, space="PSUM"))

        cons_r = singles.tile([128, NCONST_R], F32R)
        nc.sync.dma_start(out=cons_r, in_=cst_r[:])
        cons_h = singles.tile([128, 256], BF16)
        nc.sync.dma_start(out=cons_h, in_=cst_h[:])
        cons_f = singles.tile([128, NCONST_F], F32)
        nc.sync.dma_start(out=cons_f, in_=cst_f[:])

        psum_sum = psump.tile([128, CHUNK], F32)
        psum_sq = psump.tile([128, CHUNK], F32)
        psum2 = psump.tile([NG, CHUNK], F32)

        # Absorb the consts-DMA wait on the PE here so the first real matmul
        # only waits on the features DMA. (psum2 is cleared again by the
        # real start=True matmul of the MLP output group later.)
        # (2x2, not 1x1: fp32r matmuls require even free dims.)
        nc.tensor.matmul(
            psum2[0:2, 0:2],
            lhsT=cons_r[:, 0:2],
            rhs=cons_r[:, 0:2],
            start=True,
            stop=True,
        )

        # ---- streaming phase: load, square, PE column-sum reductions ----
        # One 4 MiB DMA per batch: tile [128, (C-half h), pixels].
        for b in range(B_PER_CORE):
            x = xpool.tile([128, 2, H * W], F32R, tag="x", name=f"x_{b}")
            nc.sync.dma_start(
                out=x, in_=feat[b].rearrange("(h c) p -> c h p", h=2)
            )
            # Squares cast to bf16: the squared path runs bf16 matmuls
            # (full PE clock + fast weight load); error on the positive
            # sumsq sum is ~1e-4 relative.
            sq = sqpool.tile([128, 2, H * W], BF16, tag="sq", name=f"sq_{b}")
            for half in range(2):
                nc.scalar.activation(
                    sq[:, half, :], x[:, half, :].bitcast(F32), AF.Square
                )
            for half in range(2):
                for q in range(H * W // CHUNK):  # 8 chunks per batch
                    g = b * (H * W // CHUNK) + q
                    # Full 128-column one-hot window (f32r + col-tiled output
                    # at nonzero base partition generates invalid ISA).
                    sl = slice(q * CHUNK, (q + 1) * CHUNK)
                    first = b == 0 and half == 0 and q == 0
                    last = (
                        b == B_PER_CORE - 1 and half == 1 and q == (H * W // CHUNK) - 1
                    )
                    nc.tensor.matmul(
                        psum_sum,
                        lhsT=cons_r[:, 128 - NREP * g : 256 - NREP * g],
                        rhs=x[:, half, sl],
                        start=first,
                        stop=last,
                    )
                    nc.tensor.matmul(
                        psum_sq,
                        lhsT=cons_h[:, 128 - NREP * g : 256 - NREP * g],
                        rhs=sq[:, half, sl],
                        start=first,
                        stop=last,
                    )

        # ---- stats tail, on (g, oh)-replicated [128, 512] tiles ----
        inv_c = 1.0 / C
        inv_cm1 = 1.0 / (C - 1)

        a = tailp.tile([128, CHUNK], F32)  # sum^2
        nc.scalar.activation(a, psum_sum, AF.Square)
        u = tailp.tile([128, CHUNK], F32)  # sumsq - sum^2/C
        nc.vector.scalar_tensor_tensor(
            u, in0=a, scalar=-inv_c, in1=psum_sq, op0=OP.mult, op1=OP.add
        )
        v1 = tailp.tile([128, CHUNK], F32)  # max(var, 0)
        nc.vector.tensor_scalar(
            v1, in0=u, scalar1=inv_cm1, scalar2=0.0, op0=OP.mult, op1=OP.max
        )
        var_c = tailp.tile([128, CHUNK], F32)  # clip(var, 0, 1)
        nc.vector.tensor_scalar_min(var_c, in0=v1, scalar1=1.0)
        grad = tailp.tile([128, CHUNK], F32)
        nc.scalar.activation(grad, var_c, AF.Sqrt)
        mag = tailp.tile([128, CHUNK], F32)
        nc.scalar.activation(mag, psum_sq, AF.Sqrt, scale=inv_c)
        mag_c = tailp.tile([128, CHUNK], F32)
        nc.vector.tensor_scalar_min(mag_c, in0=mag, scalar1=1.0)

        # ---- MLP: two passes over hidden halves ----
        for k in range(NPASS):
            w0 = cons_f[:, 3 * k + 0 : 3 * k + 1]
            w1 = cons_f[:, 3 * k + 1 : 3 * k + 2]
            w2 = cons_f[:, 3 * k + 2 : 3 * k + 3]
            b1c = cons_f[:, 6 + k : 7 + k]
            t0 = tailp.tile([128, CHUNK], F32, tag="t0", name=f"t0_{k}")
            nc.vector.tensor_scalar(
                t0, in0=grad, scalar1=w2, scalar2=b1c, op0=OP.mult, op1=OP.add
            )
            t1 = tailp.tile([128, CHUNK], F32, tag="t1", name=f"t1_{k}")
            nc.vector.scalar_tensor_tensor(
                t1, in0=var_c, scalar=w1, in1=t0, op0=OP.mult, op1=OP.add
            )
            t2 = tailp.tile([128, CHUNK], F32, tag="t2", name=f"t2_{k}")
            nc.vector.scalar_tensor_tensor(
                t2, in0=mag_c, scalar=w0, in1=t1, op0=OP.mult, op1=OP.add
            )
            hk = tailp.tile([128, CHUNK], F32R, tag="hk", name=f"hk_{k}")
            nc.vector.tensor_scalar_max(hk, in0=t2, scalar1=0.0)
            nc.tensor.matmul(
                psum2,
                lhsT=cons_r[:, BDCOL + NG * k : BDCOL + NG * (k + 1)],
                rhs=hk,
                start=(k == 0),
                stop=(k == NPASS - 1),
            )

        out_sb = tailp.tile([NG, CHUNK], F32)
        nc.scalar.activation(
            out_sb, psum2, AF.Sigmoid, bias=cons_f[:NG, 8:9]
        )
        nc.sync.dma_start(out=out_d[:], in_=out_sb)

    nc.finalize()
    return nc


def make_consts(W1, b1, W2, b2):
    cr = np.zeros((128, NCONST_R), np.float32)
    cr[:, 128 : 128 + NREP] = 1.0  # ones block for the windowed one-hot lhsT
    ch = np.zeros((128, 256), np.float32)
    ch[:, 128 : 128 + NREP] = 1.0  # same, bf16 flavor for the squared path
    cf = np.zeros((128, NCONST_F), np.float32)
    for g in range(NG):
        for oh in range(NREP):
            p = g * NREP + oh
            for k in range(NPASS):
                o = k * NREP + oh
                for i in range(3):
                    cf[p, k * 3 + i] = W1[i, o]
                cf[p, 6 + k] = b1[o]
                cr[p, BDCOL + k * NG + g] = W2[o, 0]
    cf[:, 8] = b2[0]
    try:
        import ml_dtypes

        ch = ch.astype(ml_dtypes.bfloat16)
    except ImportError:
        ch = ch.astype(np.uint16)  # won't happen: ml_dtypes ships with jax
    return cr, ch, cf


_CACHE: dict = {}


def _get_nc() -> bass.Bass:
    if "nc" not in _CACHE:
        _CACHE["nc"] = build_nc()
    return _CACHE["nc"]


def run_sharded(features, W1, b1, W2, b2, **spmd_kwargs):
    """Run the SPMD kernel; returns (BassKernelResults, assembled output)."""
    from concourse.bass_utils import run_bass_kernel_spmd

    feats = np.ascontiguousarray(features, dtype=np.float32).reshape(B, C, H * W)
    cr, ch, cf = make_consts(
        np.asarray(W1, np.float32),
        np.asarray(b1, np.float32),
        np.asarray(W2, np.float32),
        np.asarray(b2, np.float32),
    )
    in_maps = [
        {
            "features": np.ascontiguousarray(
                feats[r * B_PER_CORE : (r + 1) * B_PER_CORE]
            ),
            "consts_r": cr,
            "consts_h": ch,
            "consts_f": cf,
        }
        for r in range(NCORES)
    ]
    nc = _get_nc()
    res = run_bass_kernel_spmd(nc, in_maps, core_ids=list(range(NCORES)), **spmd_kwargs)
    out = np.concatenate(
        [res.results[r]["out"].reshape(B_PER_CORE, H, W) for r in range(NCORES)],
        axis=0,
    )
    return res, out


def kernel(features, W1, b1, W2, b2):
    _, out = run_sharded(features, W1, b1, W2, b2)
    return out
